# revision 1
# baseline (speedup 1.0000x reference)
"""HQQ int4 weight-only quantized linear for TRN2, 8-core tensor-parallel.

out[M, N] = x[M, K] @ dequant(W_q[N, K]).T
  dequant: w[n, k] = (q[n, k] - 8) * scales[n, k//128] + zeros[n, k//128]

Sharding: column-parallel over N (out_features) across 8 NeuronCores;
x replicated; outputs concatenated on host. No collectives.

Device algorithm per core:
  - 32 weight k-group tiles [128, n_shard] dequantized in SBUF:
    wd = (q-8) * s_bcast   (s rows replicated across partitions by GpSimd
    partition_broadcast; multiply on DVE)
  - zeros applied by zero-point compensation (standard int-GEMM trick):
    out = x @ (w8*s).T + R @ zeros.T, with R[m,g] = sum of x[m, k in g].
    The R@z.T rank-32 matmul seeds each PSUM accumulation (start=True).
  - main matmul: psum[m128, n<=512] accumulated over 32 k-tiles.
"""

import os
import sys

import numpy as np
import ml_dtypes

M = 4096
K = 4096
N = 11008
GROUP = 128
N_CORES = 8
N_SHARD = N // N_CORES  # 1376
NG = K // GROUP  # 32 quant groups == 32 k-tiles of 128
M_PANEL = 256
BF16 = ml_dtypes.bfloat16

Z_VIA_MM = True  # False -> bit-exact path (z broadcast + DVE add)


def _install_axon_hooks_shim():
    """antenv.axon_hooks is missing from this image; run_bass_kernel_spmd
    imports it when tracing is requested (e.g. BASS_TRACE=1). Provide the
    same ctypes-based hook trn_boot would have registered."""
    import types

    try:
        import antenv.axon_hooks  # noqa: F401

        return
    except ImportError:
        pass
    try:
        import antenv
        from trn_agent_boot.trn_boot import _ntff_profile_via_ctypes

        hook = _ntff_profile_via_ctypes("/opt/axon/libaxon_pjrt.so")
        mod = types.ModuleType("antenv.axon_hooks")
        mod._hook = hook
        mod.get_axon_ntff_profile_hook = lambda: mod._hook

        def _set(h):
            mod._hook = h

        mod.set_axon_ntff_profile_hook = _set
        sys.modules["antenv.axon_hooks"] = mod
        antenv.axon_hooks = mod
    except Exception:
        pass


def build_bass(m=M, k=K, n_shard=N_SHARD, ng=NG, z_via_mm=None, compile=True):
    import concourse.mybir as mybir
    import concourse.tile as tile
    from concourse import bacc

    if z_via_mm is None:
        z_via_mm = Z_VIA_MM
    P = 128
    MP = M_PANEL
    assert k == ng * GROUP and m % MP == 0 and ng % 4 == 0
    f32 = mybir.dt.float32
    bf16 = mybir.dt.bfloat16
    n_panels = m // MP
    nsub = MP // P  # m-subtiles per panel (2)

    nc = bacc.Bacc("TRN2", target_bir_lowering=False, debug=False)
    xT4 = nc.dram_tensor("xT4", [n_panels, P, ng, MP], bf16, kind="ExternalInput")
    w8 = nc.dram_tensor("w8", [k, n_shard], bf16, kind="ExternalInput")
    sT = nc.dram_tensor("sT", [ng, n_shard], bf16, kind="ExternalInput")
    zT = nc.dram_tensor("zT", [ng, n_shard], bf16, kind="ExternalInput")
    rT = nc.dram_tensor("rT", [ng, m], bf16, kind="ExternalInput")
    out = nc.dram_tensor("out", [m, n_shard], bf16, kind="ExternalOutput")

    n_tiles = []
    st = 0
    while st < n_shard:
        nf = min(512, n_shard - st)
        n_tiles.append((st, nf))
        st += nf

    GPB = ng // 4  # groups per table row (8)

    with tile.TileContext(nc) as tc:
        with (
            tc.tile_pool(name="wdeq", bufs=ng) as wdeq_pool,
            tc.tile_pool(name="small", bufs=1) as small_pool,
            tc.tile_pool(name="bc", bufs=8) as bc_pool,
            tc.tile_pool(name="xp", bufs=2) as xp_pool,
            tc.tile_pool(name="osb", bufs=2) as osb_pool,
            tc.tile_pool(name="psum", bufs=6, space="PSUM") as psum_pool,
        ):
            # ---- small tables into SBUF, zero-padded to K=128 for the
            # zero-point compensation seed matmul ----
            if z_via_mm:
                zT_sb = small_pool.tile([P, n_shard], bf16, tag="ztsb")
                nc.vector.memset(zT_sb[:], 0.0)
                nc.scalar.dma_start(zT_sb[:ng, :], zT[:, :])
                rT_sb = small_pool.tile([P, m], bf16, tag="rtsb")
                nc.vector.memset(rT_sb[:], 0.0)
                nc.scalar.dma_start(rT_sb[:ng, :], rT[:, :])

            # ---- dequant: wd = w8_tile * s_bcast (+ z_bcast if not z_via_mm) ----
            xp_tiles = {}
            wdeq_tiles = []
            for g in range(ng):
                wd = wdeq_pool.tile([P, n_shard], bf16, tag="wdeq")
                nc.sync.dma_start(wd[:], w8[g * P : (g + 1) * P, :])
                if g == 1:
                    # first x panel onto sync ring right after 2 weight tiles
                    xp_tiles[0] = xp_pool.tile([P, ng, MP], bf16, tag="xp", name="xp0")
                    nc.sync.dma_start(xp_tiles[0][:], xT4[0])
                s_bc = bc_pool.tile([P, n_shard], bf16, tag="sbc")
                ring = nc.scalar if g % 2 == 0 else nc.sync
                ring.dma_start(s_bc[:], sT[g : g + 1, :].to_broadcast((P, n_shard)))
                nc.vector.tensor_mul(wd[:], wd[:], s_bc[:])
                if not z_via_mm:
                    z_bc = bc_pool.tile([P, n_shard], bf16, tag="zbc")
                    ring.dma_start(
                        z_bc[:], zT[g : g + 1, :].to_broadcast((P, n_shard))
                    )
                    nc.vector.tensor_add(wd[:], wd[:], z_bc[:])
                wdeq_tiles.append(wd)

            # ---- matmul ----
            def seed_psum(ps, j, st, nf, ms_abs):
                if z_via_mm:
                    # zero-point compensation: psum = R_tile.T @ zT (K=32)
                    nc.tensor.matmul(
                        ps,
                        rT_sb[:, ms_abs * P : (ms_abs + 1) * P],
                        zT_sb[:, st : st + nf],
                        start=True,
                        stop=False,
                    )

            start_flag = not z_via_mm  # main MMs open the bank when no seed

            def evict(psums, ms_abs):
                osb = osb_pool.tile([P, n_shard], bf16, tag="osb")
                for j, (st, nf) in enumerate(n_tiles):
                    nc.any.tensor_copy(osb[:, st : st + nf], psums[j])
                m0 = ms_abs * P
                nc.sync.dma_start(out[m0 : m0 + P, :], osb[:])

            def emit_panel_k_outer(xp, mp):
                # all m-subtiles' k-sweeps interleaved: 6 open psum banks.
                pss = []
                for ms in range(nsub):
                    row = []
                    for j, (st, nf) in enumerate(n_tiles):
                        ps = psum_pool.tile([P, 512], f32, tag="ps", name="psA")[:, :nf]
                        seed_psum(ps, j, st, nf, mp * nsub + ms)
                        row.append(ps)
                    pss.append(row)
                for g in range(ng):
                    for ms in range(nsub):
                        lhsT = xp[:, g, ms * P : (ms + 1) * P]
                        for j, (st, nf) in enumerate(n_tiles):
                            nc.tensor.matmul(
                                pss[ms][j],
                                lhsT,
                                wdeq_tiles[g][:, st : st + nf],
                                start=(start_flag and g == 0),
                                stop=(g == ng - 1),
                            )
                for ms in range(nsub):
                    evict(pss[ms], mp * nsub + ms)

            def emit_panel_ms_inner(xp, mp):
                for ms in range(nsub):
                    psums = []
                    for j, (st, nf) in enumerate(n_tiles):
                        ps = psum_pool.tile([P, 512], f32, tag="ps", name="psB")[:, :nf]
                        seed_psum(ps, j, st, nf, mp * nsub + ms)
                        psums.append(ps)
                    for g in range(ng):
                        lhsT = xp[:, g, ms * P : (ms + 1) * P]
                        for j, (st, nf) in enumerate(n_tiles):
                            nc.tensor.matmul(
                                psums[j],
                                lhsT,
                                wdeq_tiles[g][:, st : st + nf],
                                start=(start_flag and g == 0),
                                stop=(g == ng - 1),
                            )
                    evict(psums, mp * nsub + ms)

            for mp in range(n_panels):
                if mp not in xp_tiles:
                    xp_tiles[mp] = xp_pool.tile(
                        [P, ng, MP], bf16, tag="xp", name=f"xp{mp}"
                    )
                    nc.sync.dma_start(xp_tiles[mp][:], xT4[mp])
                if mp < 3:
                    emit_panel_k_outer(xp_tiles[mp], mp)
                else:
                    emit_panel_ms_inner(xp_tiles[mp], mp)

    if compile:
        nc.compile()
    return nc


def host_prep(x, W_q, scales, zeros, m=M, k=K, ng=NG):
    """Shared host-side layout prep. Returns full-size tensors to shard."""
    n = W_q.shape[0]
    nsh = n // N_CORES
    x = np.asarray(x)
    xf = x.astype(np.float32)
    n_panels = m // M_PANEL
    # x tiled: [panel, ki, ko, m_in_panel]
    xT4 = np.ascontiguousarray(
        x.reshape(n_panels, M_PANEL, ng, GROUP).transpose(0, 3, 2, 1)
    )
    # per-group row sums of x (zero-point compensation operand)
    rT = np.ascontiguousarray(
        xf.reshape(m, ng, GROUP).sum(-1).T.astype(BF16)
    )  # [ng, m]
    w8_full = np.ascontiguousarray(
        (np.asarray(W_q).astype(np.float32) - 8.0).astype(BF16).T
    )  # [K, N]
    sT_full = np.ascontiguousarray(np.asarray(scales).astype(BF16, copy=False).T)
    zT_full = np.ascontiguousarray(np.asarray(zeros).astype(BF16, copy=False).T)
    return xT4, rT, w8_full, sT_full, zT_full, nsh


def interleave_tab(s_c, z_c, ng):
    """[ng, ns] s/z -> [4, 2*(ng//4)*ns] table: row r holds groups g%4==r."""
    gpb = ng // 4
    ns = s_c.shape[1]

    def il(a):
        return a.reshape(gpb, 4, ns).transpose(1, 0, 2).reshape(4, gpb * ns)

    return np.ascontiguousarray(np.concatenate([il(s_c), il(z_c)], axis=1))


_NC_CACHE = {}
_LAST_IN_MAPS = None


def kernel(x, W_q, scales, zeros):
    _install_axon_hooks_shim()
    from concourse.bass_utils import run_bass_kernel_spmd

    xT4, rT, w8_full, sT_full, zT_full, nsh = host_prep(x, W_q, scales, zeros)
    assert nsh == N_SHARD

    if "nc" not in _NC_CACHE:
        _NC_CACHE["nc"] = build_bass()
    nc = _NC_CACHE["nc"]

    in_maps = []
    for c in range(N_CORES):
        lo, hi = c * N_SHARD, (c + 1) * N_SHARD
        s_c = sT_full[:, lo:hi]
        z_c = zT_full[:, lo:hi]
        in_maps.append(
            {
                "xT4": xT4,
                "w8": np.ascontiguousarray(w8_full[:, lo:hi]),
                "sT": np.ascontiguousarray(s_c),
                "zT": np.ascontiguousarray(z_c),
                "rT": rT,
            }
        )

    global _LAST_IN_MAPS
    _LAST_IN_MAPS = in_maps
    res = run_bass_kernel_spmd(nc, in_maps, list(range(N_CORES)))
    out = np.concatenate([res.results[c]["out"] for c in range(N_CORES)], axis=1)
    return out.astype(BF16, copy=False)



# revision 2
# speedup vs baseline: 1.0441x; 1.0441x over previous
"""HQQ int4 weight-only quantized linear for TRN2, 8-core tensor-parallel.

out[M, N] = x[M, K] @ dequant(W_q[N, K]).T
  dequant: w[n, k] = (q[n, k] - 8) * scales[n, k//128] + zeros[n, k//128]

Sharding: column-parallel over N (out_features) across 8 NeuronCores;
x replicated; outputs concatenated on host. No collectives.

v2: weights are fully dequantized on the host (fp32 math, bf16 result)
and shipped as wT[K, n_shard] per core, so the device does matmul only:
  - 32 k-group weight tiles [128, n_shard] DMA'd straight to SBUF
  - per m-subtile of 128 rows: 3 PSUM banks (512/512/352 cols),
    accumulated over the 32 k-tiles, evicted per-bank to SBUF + HBM
  - PE warmup burst (dummy matmuls on zeros) during the initial DMA wait
    so real matmuls run at the warm 2.4 GHz clock from the start
  - panel-0 x DMA split into 4 chunks so the first matmul starts ~4us in
"""

import os
import sys

import numpy as np
import ml_dtypes

M = 4096
K = 4096
N = 11008
GROUP = 128
N_CORES = 8
N_SHARD = N // N_CORES  # 1376
NG = K // GROUP  # 32 quant groups == 32 k-tiles of 128
M_PANEL = 256
BF16 = ml_dtypes.bfloat16


def _install_axon_hooks_shim():
    """antenv.axon_hooks is missing from this image; run_bass_kernel_spmd
    imports it when tracing is requested (e.g. BASS_TRACE=1). Provide the
    same ctypes-based hook trn_boot would have registered."""
    import types

    try:
        import antenv.axon_hooks  # noqa: F401

        return
    except ImportError:
        pass
    try:
        import antenv
        from trn_agent_boot.trn_boot import _ntff_profile_via_ctypes

        hook = _ntff_profile_via_ctypes("/opt/axon/libaxon_pjrt.so")
        mod = types.ModuleType("antenv.axon_hooks")
        mod._hook = hook
        mod.get_axon_ntff_profile_hook = lambda: mod._hook

        def _set(h):
            mod._hook = h

        mod.set_axon_ntff_profile_hook = _set
        sys.modules["antenv.axon_hooks"] = mod
        antenv.axon_hooks = mod
    except Exception:
        pass


def build_bass(m=M, k=K, n_shard=N_SHARD, ng=NG, compile=True):
    import concourse.mybir as mybir
    import concourse.tile as tile
    from concourse import bacc

    P = 128
    MP = M_PANEL
    assert k == ng * GROUP and m % MP == 0
    f32 = mybir.dt.float32
    bf16 = mybir.dt.bfloat16
    n_panels = m // MP
    nsub = MP // P  # m-subtiles per panel (2)

    nc = bacc.Bacc("TRN2", target_bir_lowering=False, debug=False)
    xT4 = nc.dram_tensor("xT4", [n_panels, P, ng, MP], bf16, kind="ExternalInput")
    wT = nc.dram_tensor("wT", [k, n_shard], bf16, kind="ExternalInput")
    out = nc.dram_tensor("out", [m, n_shard], bf16, kind="ExternalOutput")

    n_tiles = []
    st = 0
    while st < n_shard:
        nf = min(512, n_shard - st)
        n_tiles.append((st, nf))
        st += nf

    with tile.TileContext(nc) as tc:
        with (
            tc.tile_pool(name="wdeq", bufs=ng) as wdeq_pool,
            tc.tile_pool(name="warm", bufs=1) as warm_pool,
            tc.tile_pool(name="xp", bufs=3) as xp_pool,
            tc.tile_pool(name="osb", bufs=2) as osb_pool,
            tc.tile_pool(name="psum", bufs=6, space="PSUM") as psum_pool,
            tc.tile_pool(name="pwarm", bufs=1, space="PSUM") as pwarm_pool,
        ):
            # ---- PE warmup: ~3.4us of dummy matmuls flips the HAM clock
            # gate to 8/8 while the first DMAs are still in flight ----
            wz = warm_pool.tile([P, 512], bf16, tag="wz")
            nc.vector.memset(wz[:], 0.0)
            pw = pwarm_pool.tile([P, 512], f32, tag="pw")
            for _ in range(8):
                nc.tensor.matmul(pw, wz[:, :P], wz[:], start=True, stop=True)

            # ---- weight tiles straight to SBUF (sync queue) ----
            wdeq_tiles = []
            for g in range(ng):
                wd = wdeq_pool.tile([P, n_shard], bf16, tag="wdeq")
                nc.sync.dma_start(wd[:], wT[g * P : (g + 1) * P, :])
                wdeq_tiles.append(wd)

            # ---- panel-0 x in 4 chunks (scalar queue) so chunk 0 lands
            # fast and the first matmul isn't gated on a 2MB transfer ----
            xp_tiles = {}
            xp_tiles[0] = xp_pool.tile([P, ng, MP], bf16, tag="xp", name="xp0")
            gchunk = ng // 4
            for c in range(4):
                sl = slice(c * gchunk, (c + 1) * gchunk)
                nc.scalar.dma_start(xp_tiles[0][:, sl, :], xT4[0][:, sl, :])

            def evict(psums, ms_abs):
                osb = osb_pool.tile([P, n_shard], bf16, tag="osb")
                m0 = ms_abs * P
                for j, (st, nf) in enumerate(n_tiles):
                    nc.any.tensor_copy(osb[:, st : st + nf], psums[j])
                    nc.sync.dma_start(
                        out[m0 : m0 + P, st : st + nf], osb[:, st : st + nf]
                    )

            def emit_panel_k_outer(xp, mp):
                # both m-subtiles' k-sweeps interleaved: 6 open psum banks;
                # halves the weight-DMA rate needed while weights stream in.
                pss = []
                for ms in range(nsub):
                    row = []
                    for j, (st, nf) in enumerate(n_tiles):
                        ps = psum_pool.tile([P, 512], f32, tag="ps", name="psA")[:, :nf]
                        row.append(ps)
                    pss.append(row)
                for g in range(ng):
                    for ms in range(nsub):
                        lhsT = xp[:, g, ms * P : (ms + 1) * P]
                        for j, (st, nf) in enumerate(n_tiles):
                            nc.tensor.matmul(
                                pss[ms][j],
                                lhsT,
                                wdeq_tiles[g][:, st : st + nf],
                                start=(g == 0),
                                stop=(g == ng - 1),
                            )
                for ms in range(nsub):
                    evict(pss[ms], mp * nsub + ms)

            def emit_panel_ms_inner(xp, mp):
                for ms in range(nsub):
                    psums = []
                    for j, (st, nf) in enumerate(n_tiles):
                        ps = psum_pool.tile([P, 512], f32, tag="ps", name="psB")[:, :nf]
                        psums.append(ps)
                    for g in range(ng):
                        lhsT = xp[:, g, ms * P : (ms + 1) * P]
                        for j, (st, nf) in enumerate(n_tiles):
                            nc.tensor.matmul(
                                psums[j],
                                lhsT,
                                wdeq_tiles[g][:, st : st + nf],
                                start=(g == 0),
                                stop=(g == ng - 1),
                            )
                    evict(psums, mp * nsub + ms)

            for mp in range(n_panels):
                # keep 2 panels of x prefetch in flight
                for q in (mp + 1, mp + 2):
                    if q < n_panels and q not in xp_tiles:
                        xp_tiles[q] = xp_pool.tile(
                            [P, ng, MP], bf16, tag="xp", name=f"xp{q}"
                        )
                        nc.scalar.dma_start(xp_tiles[q][:], xT4[q])
                if mp < 2:
                    emit_panel_k_outer(xp_tiles[mp], mp)
                else:
                    emit_panel_ms_inner(xp_tiles[mp], mp)

    if compile:
        nc.compile()
    return nc


def host_prep(x, W_q, scales, zeros):
    """Host-side prep: x tiled for the kernel layout; weights fully
    dequantized in fp32 and transposed to [K, N] bf16."""
    x = np.asarray(x)
    n_panels = M // M_PANEL
    # x tiled: [panel, k_in_group, group, m_in_panel]
    xT4 = np.ascontiguousarray(
        x.reshape(n_panels, M_PANEL, NG, GROUP).transpose(0, 3, 2, 1)
    )
    q = np.asarray(W_q).astype(np.float32).reshape(N, NG, GROUP)
    s = np.asarray(scales).astype(np.float32)[:, :, None]
    z = np.asarray(zeros).astype(np.float32)[:, :, None]
    w = ((q - 8.0) * s + z).astype(BF16).reshape(N, K)  # [N, K]
    wT_full = np.ascontiguousarray(w.T)  # [K, N]
    return xT4, wT_full


_NC_CACHE = {}
_LAST_IN_MAPS = None


def kernel(x, W_q, scales, zeros):
    _install_axon_hooks_shim()
    from concourse.bass_utils import run_bass_kernel_spmd

    xT4, wT_full = host_prep(x, W_q, scales, zeros)

    if "nc" not in _NC_CACHE:
        _NC_CACHE["nc"] = build_bass()
    nc = _NC_CACHE["nc"]

    in_maps = []
    for c in range(N_CORES):
        lo, hi = c * N_SHARD, (c + 1) * N_SHARD
        in_maps.append(
            {
                "xT4": xT4,
                "wT": np.ascontiguousarray(wT_full[:, lo:hi]),
            }
        )

    global _LAST_IN_MAPS
    _LAST_IN_MAPS = in_maps
    res = run_bass_kernel_spmd(nc, in_maps, list(range(N_CORES)))
    out = np.concatenate([res.results[c]["out"] for c in range(N_CORES)], axis=1)
    return out.astype(BF16, copy=False)
